# revision 29
# baseline (speedup 1.0000x reference)
"""Masked mean neighbor aggregation (GNN message passing) on 8 TRN2 cores.

Strategy (per spec sharding hint): batch dim sharded across 8 cores, feature
table replicated in each core's DRAM (converted to bf16 on the host, which
halves gather traffic and runs the PE at bf16 rate).

Device algorithm, per core, per 128-row output tile:
  - The table is split into 4 row-range chunks (<=32768 rows each) so row
    indices fit dma_gather's int16 index format.  The host compacts each
    tile's unmasked (row, neighbor) slots by chunk into fixed-capacity flat
    index lists.
  - 4 dma_gather instructions (one per chunk, on 4 parallel SWDGE queues)
    pull the 256 B bf16 feature rows into an SBUF tile G.  Trailing slots
    are -1 (dma_gather skips them), so only the group's real slot count is
    transferred.  The SWDGE ring accounting is computed from num_idxs_reg,
    which must equal the exact count of non-negative indices and is baked
    into the instruction stream -- so the count of each (tile, chunk) group
    is made uniform across the 8 SPMD cores by padding up to the max with
    dummy valid slots (row 0, dead selection lane).
  - The slots land in a data-dependent order, so the host also ships a tiny
    per-slot target-lane array; the vector engine expands it on device into
    one-hot bf16 selection matrices (pad slots get lane 128 -> all-zero
    column).
  - PE computes psum[b, d] = sum_slots sel[slot, b] * G[slot, d] with PSUM
    accumulation over the 16 slot-blocks: simultaneously the masked sum and
    the reordering.
  - The scalar engine scales by 1/max(count,1) (host-precomputed) and the
    f32 result is stored.

The first DEPTH tiles gather a full cap so the G ring buffers never expose
uninitialized SBUF (stale NaN * 0 = NaN in PSUM).  Everything is raw bacc
with manual semaphores (the Tile layer does not know dma_gather's DMA
semantics).
"""

from contextlib import ExitStack

import numpy as np

N_NODES = 100000
D_FEAT = 128
BATCH = 50000
K = 25
N_CORES = 8
P = 128

N_CHUNKS = 4
CHUNK_ROWS = 25000           # N_NODES / N_CHUNKS, < 32768 so int16-safe
CAP = 512                    # per (tile, chunk) gather capacity, mult of 128
TILES_PER_CORE = 49          # ceil(50000 / 8 / 128)
B_LOC = TILES_PER_CORE * P   # 6272
B_PAD = B_LOC * N_CORES      # 50176
DEPTH = 8                    # ring depth; first DEPTH tiles gather full cap

_prog_cache = {}


def _build_program(n_rows, chunk_rows, n_chunks, d, n_tiles, cap, counts,
                   reps=1):
    import concourse.bass as bass
    import concourse.bacc as bacc
    import concourse.mybir as mybir
    from concourse.library_config import mlp

    bpc = cap // P               # G column-blocks per chunk
    nblk = n_chunks * bpc        # selection blocks per tile
    ic = cap // 16               # idx columns per gather (wrapped int16)

    nc = bacc.Bacc("TRN2", target_bir_lowering=False, debug=False,
                   num_devices=N_CORES, num_swdge_queues=n_chunks)

    ftab = nc.dram_tensor("features", [n_rows, d], mybir.dt.bfloat16,
                          kind="ExternalInput")
    # idx ships unreplicated (16 partitions); the ucode wants it replicated
    # x8 across 128 partitions, which is done on device with 3 doubling
    # SBUF->SBUF copies (saves 1.4 MB of HBM reads per pass).
    idx_d = nc.dram_tensor("idx", [16, n_tiles * n_chunks * ic],
                           mybir.dt.int16, kind="ExternalInput")
    b_d = nc.dram_tensor("bidx", [P, n_tiles * nblk], mybir.dt.bfloat16,
                         kind="ExternalInput")
    iota_d = nc.dram_tensor("iota", [P, P], mybir.dt.bfloat16,
                            kind="ExternalInput")
    winv_d = nc.dram_tensor("winv", [P, n_tiles], mybir.dt.float32,
                            kind="ExternalInput")
    out_d = nc.dram_tensor("out", [n_tiles * P, d], mybir.dt.bfloat16,
                           kind="ExternalOutput")

    with ExitStack() as stack:
        block = stack.enter_context(nc.Block())
        ec = stack.enter_context
        idx_sb = ec(nc.sbuf_tensor("idx_sb", [P, n_tiles * n_chunks * ic],
                                   mybir.dt.int16))
        b_sb = ec(nc.sbuf_tensor("b_sb", [P, n_tiles * nblk],
                                 mybir.dt.bfloat16))
        iota_sb = ec(nc.sbuf_tensor("iota_sb", [P, P], mybir.dt.bfloat16))
        winv_sb = ec(nc.sbuf_tensor("winv_sb", [P, n_tiles],
                                    mybir.dt.float32))
        G = [ec(nc.sbuf_tensor(f"g{r}", [P, nblk, d], mybir.dt.bfloat16))
             for r in range(DEPTH)]
        SEL = [ec(nc.sbuf_tensor(f"sel{r}", [P, nblk, P], mybir.dt.bfloat16))
               for r in range(DEPTH)]
        OSB = [ec(nc.sbuf_tensor(f"osb{r}", [P, d], mybir.dt.bfloat16))
               for r in range(DEPTH)]
        PS = [ec(nc.psum_tensor(f"ps{r}", [P, d], mybir.dt.float32))
              for r in range(DEPTH)]
        r0 = ec(nc.semaphore("r0"))
        r1 = ec(nc.semaphore("r1"))
        io_vec = ec(nc.semaphore("io_vec"))
        io_w = ec(nc.semaphore("io_w"))
        gq = [[ec(nc.semaphore(f"gq{c}_{r}")) for r in range(DEPTH)]
              for c in range(n_chunks)]
        selg = ec(nc.semaphore("selg"))
        mmd = ec(nc.semaphore("mmd"))
        scd = ec(nc.semaphore("scd"))
        sto = [ec(nc.semaphore(f"sto{r}")) for r in range(DEPTH)]

        split = min(DEPTH, n_tiles)
        c0 = split * n_chunks * ic          # idx cols for the first tiles
        cA = n_tiles * n_chunks * ic

        @block.sync
        def _(sync: bass.BassEngine):
            # idx piece for the first `split` tiles lands first so gathers
            # start ~4 us earlier than a monolithic idx DMA would allow
            sync.dma_start(idx_sb[0:16, 0:c0], idx_d[:, 0:c0]).then_inc(
                r0, 16)
            sync.dma_start(b_sb[:], b_d[:]).then_inc(io_vec, 16)
            sync.dma_start(iota_sb[:], iota_d[:]).then_inc(io_vec, 16)
            sync.dma_start(winv_sb[:], winv_d[:]).then_inc(io_w, 16)
            sync.dma_start(idx_sb[0:16, c0:cA], idx_d[:, c0:cA]).then_inc(
                r1, 16)
            # replicate 16 -> 128 partitions by doubling (ucode reads the
            # wrapped idx replicated x8)
            for rng, sem in (((0, c0), r0), ((c0, cA), r1)):
                a, b = rng
                sync.wait_ge(sem, 16)
                sync.dma_start(idx_sb[16:32, a:b],
                               idx_sb[0:16, a:b]).then_inc(sem, 16)
                sync.wait_ge(sem, 32)
                sync.dma_start(idx_sb[32:64, a:b],
                               idx_sb[0:32, a:b]).then_inc(sem, 16)
                sync.wait_ge(sem, 48)
                sync.dma_start(idx_sb[64:128, a:b],
                               idx_sb[0:64, a:b]).then_inc(sem, 16)
            for tau in range(reps * n_tiles):
                t = tau % n_tiles
                sync.wait_ge(scd, tau + 1)
                sync.dma_start(out_d[t * P:(t + 1) * P, :],
                               OSB[tau % DEPTH][:]).then_inc(
                                   sto[tau % DEPTH], 16)

        @block.gpsimd
        def _(gpsimd: bass.BassGpSimd):
            gpsimd.load_library(mlp)
            gpsimd.wait_ge(r0, 64)
            for tau in range(reps * n_tiles):
                t = tau % n_tiles
                if tau == split:
                    gpsimd.wait_ge(r1, 64)
                if tau >= DEPTH:
                    gpsimd.wait_ge(mmd, tau - DEPTH + 1)  # G[tau%DEPTH] free
                gt = G[tau % DEPTH]
                for c in range(n_chunks):
                    src = ftab[c * chunk_rows:(c + 1) * chunk_rows, :]
                    idxs = idx_sb[:, (t * n_chunks + c) * ic:
                                  (t * n_chunks + c + 1) * ic]
                    gpsimd.dma_gather(
                        gt[:, c * bpc:(c + 1) * bpc, :], src, idxs,
                        cap, int(counts[t * n_chunks + c]), d, queue_num=c,
                    ).then_inc(gq[c][tau % DEPTH], 16)

        @block.vector
        def _(vector: bass.BassVectorEngine):
            vector.wait_ge(io_vec, 32)  # b + iota landed
            iv = iota_sb.ap()
            iota_bc = bass.AP(iv.tensor, iv.offset,
                              [iv.ap[0], [0, nblk], iv.ap[1]])
            for tau in range(reps * n_tiles):
                t = tau % n_tiles
                if tau >= DEPTH:
                    vector.wait_ge(mmd, tau - DEPTH + 1)  # sel free
                st = SEL[tau % DEPTH]
                bv = b_sb[:, t * nblk:(t + 1) * nblk]
                b_bc = bass.AP(bv.tensor, bv.offset,
                               [bv.ap[0], bv.ap[1], [0, P]])
                vector.tensor_tensor(
                    out=st[:], in0=iota_bc, in1=b_bc,
                    op=mybir.AluOpType.is_equal,
                ).then_inc(selg, 1)

        @block.scalar
        def _(scalar: bass.BassEngine):
            scalar.wait_ge(io_w, 16)
            for tau in range(reps * n_tiles):
                t = tau % n_tiles
                scalar.wait_ge(mmd, tau + 1)     # psum[tau%DEPTH] ready
                if tau >= DEPTH:
                    scalar.wait_ge(sto[tau % DEPTH],
                                   16 * (tau // DEPTH))  # OSB free
                scalar.mul(OSB[tau % DEPTH][:], PS[tau % DEPTH][:],
                           winv_sb[:, t:t + 1]).then_inc(scd, 1)

        @block.tensor
        def _(tensor: bass.BassEngine):
            for tau in range(reps * n_tiles):
                for c in range(n_chunks):
                    tensor.wait_ge(gq[c][tau % DEPTH],
                                   16 * (tau // DEPTH + 1))
                tensor.wait_ge(selg, tau + 1)
                if tau >= DEPTH:
                    tensor.wait_ge(scd, tau - DEPTH + 1)  # psum drained
                gt, st, pst = (G[tau % DEPTH], SEL[tau % DEPTH],
                               PS[tau % DEPTH])
                for blk in range(nblk):
                    inst = nc.tensor.matmul(
                        pst[:], st[:, blk, :], gt[:, blk, :],
                        start=(blk == 0), stop=(blk == nblk - 1),
                    )
                inst.then_inc(mmd, 1)

    nc.compile()
    return nc


def get_program(counts, reps=1):
    key = (N_NODES, CHUNK_ROWS, N_CHUNKS, D_FEAT, TILES_PER_CORE, CAP,
           counts.tobytes(), reps)
    if key not in _prog_cache:
        _prog_cache[key] = _build_program(
            N_NODES, CHUNK_ROWS, N_CHUNKS, D_FEAT, TILES_PER_CORE, CAP,
            counts, reps=reps)
    return _prog_cache[key]


def _group_slots(midx, mask, n_tiles, chunk_rows, n_chunks):
    """Sort one core's unmasked slots by (tile, chunk); return flat arrays
    plus group start/end offsets."""
    bb, kk = np.nonzero(mask)
    gidx = midx[bb, kk]
    tile = bb // P
    lane = bb % P
    chunk = gidx // chunk_rows
    local = (gidx % chunk_rows).astype(np.int16)
    order = np.lexsort((chunk, tile))
    lane, chunk, local = lane[order], chunk[order], local[order]
    gkey = tile[order] * n_chunks + chunk
    starts = np.searchsorted(gkey, np.arange(n_tiles * n_chunks))
    ends = np.searchsorted(gkey, np.arange(n_tiles * n_chunks) + 1)
    return local, lane, starts, ends


def pack_core(local, lane, starts, ends, counts, n_tiles, cap, n_chunks):
    """Build one core's gather-index / selection-lane arrays with the
    cross-core-uniform per-group counts."""
    import ml_dtypes
    bpc = cap // P
    nblk = n_chunks * bpc
    ic = cap // 16

    idx_arr = np.zeros((16, n_tiles * n_chunks * ic), np.int16)
    b_arr = np.full((P, n_tiles * nblk), float(P), ml_dtypes.bfloat16)

    for g in range(n_tiles * n_chunks):
        s, e = starts[g], ends[g]
        n = e - s
        # The SWDGE ring accounting derived from num_idxs_reg (= counts[g],
        # the max across cores) only depends on ceil(count/128), so each
        # core may trim to its own exact count as long as it stays in the
        # same 128-bucket: pad up to the bucket floor with dummy valid
        # slots (row 0, dead lane), then -1 (skipped by the DMA ucode).
        # The first DEPTH tiles gather the full uniform count so the G
        # ring buffers are completely initialized.
        bucket_floor = ((int(counts[g]) + 127) // 128 - 1) * 128 + 1
        if g < DEPTH * n_chunks:
            cnt = int(counts[g])
        else:
            cnt = max(n, bucket_floor)
        assert n <= cnt <= int(counts[g]) <= cap
        flat_idx = np.full(cap, -1, np.int16)
        flat_idx[:cnt] = 0
        flat_idx[:n] = local[s:e]
        flat_b = np.full(cap, float(P), np.float32)
        flat_b[:n] = lane[s:e]
        # wrapped int16 layout: flat j -> [j%16, j//16]; the x8 partition
        # replication happens on device
        idx_arr[:, g * ic:(g + 1) * ic] = flat_idx.reshape(ic, 16).T
        # selection lane values: flat j -> block j//128, partition j%128
        t, c = g // n_chunks, g % n_chunks
        cols = flat_b.reshape(bpc, P).T          # [P, bpc]
        b_arr[:, t * nblk + c * bpc:(t * nblk + (c + 1) * bpc)] = cols
    return idx_arr, b_arr


def prep_inputs(features, neigh_idx, neigh_mask):
    import ml_dtypes
    features = np.ascontiguousarray(
        np.asarray(features, dtype=np.float32).astype(ml_dtypes.bfloat16))
    neigh_idx = np.asarray(neigh_idx).astype(np.int64)
    neigh_mask = np.asarray(neigh_mask).astype(bool)

    winv = (1.0 / np.maximum(neigh_mask.sum(-1), 1)).astype(np.float32)

    pad = B_PAD - BATCH
    midx = np.concatenate(
        [neigh_idx, np.zeros((pad, K), np.int64)], axis=0).astype(np.int32)
    mask = np.concatenate([neigh_mask, np.zeros((pad, K), bool)], axis=0)
    winv = np.concatenate([winv, np.ones(pad, np.float32)])

    iota = np.tile(np.arange(P, dtype=np.float32), (P, 1)).astype(
        ml_dtypes.bfloat16)

    n_groups = TILES_PER_CORE * N_CHUNKS
    grouped = []
    counts = np.zeros(n_groups, np.int32)
    for c in range(N_CORES):
        sl = slice(c * B_LOC, (c + 1) * B_LOC)
        local, lane, starts, ends = _group_slots(
            midx[sl], mask[sl], TILES_PER_CORE, CHUNK_ROWS, N_CHUNKS)
        grouped.append((local, lane, starts, ends))
        counts = np.maximum(counts, (ends - starts).astype(np.int32))
    # num_idxs_reg must be >= 1 (a zero-index gather wedges the ucode), and
    # the first DEPTH tiles must gather a full cap so the G ring buffers
    # never expose uninitialized SBUF.
    counts = np.maximum(counts, 1)
    counts[:DEPTH * N_CHUNKS] = CAP
    assert counts.max() <= CAP

    in_maps = []
    for c in range(N_CORES):
        local, lane, starts, ends = grouped[c]
        idx_arr, b_arr = pack_core(local, lane, starts, ends, counts,
                                   TILES_PER_CORE, CAP, N_CHUNKS)
        sl = slice(c * B_LOC, (c + 1) * B_LOC)
        winv_arr = np.ascontiguousarray(
            winv[sl].reshape(TILES_PER_CORE, P).T.astype(np.float32))
        in_maps.append({
            "features": features,
            "idx": idx_arr,
            "bidx": b_arr,
            "iota": iota,
            "winv": winv_arr,
        })
    return in_maps, counts


def kernel(features, neigh_idx, neigh_mask):
    from concourse.bass_utils import run_bass_kernel_spmd

    in_maps, counts = prep_inputs(features, neigh_idx, neigh_mask)
    nc = get_program(counts)
    res = run_bass_kernel_spmd(nc, in_maps, list(range(N_CORES)))
    full = np.concatenate(
        [res.results[c]["out"].astype(np.float32) for c in range(N_CORES)],
        axis=0)
    return full[:BATCH]


# revision 33
# speedup vs baseline: 1.0865x; 1.0865x over previous
"""Masked mean neighbor aggregation (GNN message passing) on 8 TRN2 cores.

Strategy (per spec sharding hint): batch dim sharded across 8 cores, feature
table replicated in each core's DRAM (converted to bf16 on the host, which
halves gather traffic and runs the PE at bf16 rate).

Device algorithm, per core, per 128-row output tile:
  - The table is split into 4 row-range chunks (<=32768 rows each) so row
    indices fit dma_gather's int16 index format.  The host compacts each
    tile's unmasked (row, neighbor) slots by chunk into fixed-capacity flat
    index lists.
  - 4 dma_gather instructions (one per chunk, on 4 parallel SWDGE queues)
    pull the 256 B bf16 feature rows into an SBUF tile G.  Trailing slots
    are -1 (dma_gather skips them), so only the group's real slot count is
    transferred.  The SWDGE ring accounting is computed from num_idxs_reg,
    which must equal the exact count of non-negative indices and is baked
    into the instruction stream -- so the count of each (tile, chunk) group
    is made uniform across the 8 SPMD cores by padding up to the max with
    dummy valid slots (row 0, dead selection lane).
  - The slots land in a data-dependent order, so the host also ships a tiny
    per-slot target-lane array; the vector engine expands it on device into
    one-hot bf16 selection matrices (pad slots get lane 128 -> all-zero
    column).
  - PE computes psum[b, d] = sum_slots sel[slot, b] * G[slot, d] with PSUM
    accumulation over the 16 slot-blocks: simultaneously the masked sum and
    the reordering.
  - The scalar engine scales by 1/max(count,1) (host-precomputed) and the
    f32 result is stored.

The first DEPTH tiles gather a full cap so the G ring buffers never expose
uninitialized SBUF (stale NaN * 0 = NaN in PSUM).  Everything is raw bacc
with manual semaphores (the Tile layer does not know dma_gather's DMA
semantics).
"""

from contextlib import ExitStack

import numpy as np

N_NODES = 100000
D_FEAT = 128
BATCH = 50000
K = 25
N_CORES = 8
P = 128

N_CHUNKS = 4
CHUNK_ROWS = 25000           # N_NODES / N_CHUNKS, < 32768 so int16-safe
CAP = 512                    # per (tile, chunk) gather capacity, mult of 128
TILES_PER_CORE = 49          # ceil(50000 / 8 / 128)
B_LOC = TILES_PER_CORE * P   # 6272
B_PAD = B_LOC * N_CORES      # 50176
DEPTH = 8                    # ring depth; first DEPTH tiles gather full cap

_prog_cache = {}


def _build_program(n_rows, chunk_rows, n_chunks, d, n_tiles, cap, counts,
                   reps=1):
    import concourse.bass as bass
    import concourse.bacc as bacc
    import concourse.mybir as mybir
    from concourse.library_config import mlp

    bpc = cap // P               # G column-blocks per chunk
    nblk = n_chunks * bpc        # selection blocks per tile
    ic = cap // 16               # idx columns per gather (wrapped int16)

    nc = bacc.Bacc("TRN2", target_bir_lowering=False, debug=False,
                   num_devices=N_CORES, num_swdge_queues=n_chunks)

    ftab = nc.dram_tensor("features", [n_rows, d], mybir.dt.bfloat16,
                          kind="ExternalInput")
    # idx ships unreplicated (16 partitions); the ucode wants it replicated
    # x8 across 128 partitions, which is done on device with 3 doubling
    # SBUF->SBUF copies (saves 1.4 MB of HBM reads per pass).
    idx_d = nc.dram_tensor("idx", [16, n_tiles * n_chunks * ic],
                           mybir.dt.int16, kind="ExternalInput")
    b_d = nc.dram_tensor("bidx", [P, n_tiles * nblk], mybir.dt.uint8,
                         kind="ExternalInput")
    iota_d = nc.dram_tensor("iota", [P, P], mybir.dt.uint8,
                            kind="ExternalInput")
    winv_d = nc.dram_tensor("winv", [P, n_tiles], mybir.dt.float32,
                            kind="ExternalInput")
    out_d = nc.dram_tensor("out", [n_tiles * P, d], mybir.dt.bfloat16,
                           kind="ExternalOutput")

    with ExitStack() as stack:
        block = stack.enter_context(nc.Block())
        ec = stack.enter_context
        idx_sb = ec(nc.sbuf_tensor("idx_sb", [P, n_tiles * n_chunks * ic],
                                   mybir.dt.int16))
        b_sb = ec(nc.sbuf_tensor("b_sb", [P, n_tiles * nblk],
                                 mybir.dt.uint8))
        iota_sb = ec(nc.sbuf_tensor("iota_sb", [P, P], mybir.dt.uint8))
        winv_sb = ec(nc.sbuf_tensor("winv_sb", [P, n_tiles],
                                    mybir.dt.float32))
        G = [ec(nc.sbuf_tensor(f"g{r}", [P, nblk, d], mybir.dt.bfloat16))
             for r in range(DEPTH)]
        SEL = [ec(nc.sbuf_tensor(f"sel{r}", [P, nblk, P], mybir.dt.bfloat16))
               for r in range(DEPTH)]
        OSB = [ec(nc.sbuf_tensor(f"osb{r}", [P, d], mybir.dt.bfloat16))
               for r in range(DEPTH)]
        PS = [ec(nc.psum_tensor(f"ps{r}", [P, d], mybir.dt.float32))
              for r in range(DEPTH)]
        r0 = ec(nc.semaphore("r0"))
        r1 = ec(nc.semaphore("r1"))
        io_vec = ec(nc.semaphore("io_vec"))
        io_w = ec(nc.semaphore("io_w"))
        gq = [[ec(nc.semaphore(f"gq{c}_{r}")) for r in range(DEPTH)]
              for c in range(n_chunks)]
        selg = ec(nc.semaphore("selg"))
        mmd = ec(nc.semaphore("mmd"))
        scd = ec(nc.semaphore("scd"))
        sto = [ec(nc.semaphore(f"sto{r}")) for r in range(DEPTH)]

        split = min(DEPTH, n_tiles)
        c0 = split * n_chunks * ic          # idx cols for the first tiles
        cA = n_tiles * n_chunks * ic

        @block.sync
        def _(sync: bass.BassEngine):
            # idx piece for the first `split` tiles lands first so gathers
            # start ~4 us earlier than a monolithic idx DMA would allow
            sync.dma_start(idx_sb[0:16, 0:c0], idx_d[:, 0:c0]).then_inc(
                r0, 16)
            sync.dma_start(b_sb[:], b_d[:]).then_inc(io_vec, 16)
            sync.dma_start(iota_sb[:], iota_d[:]).then_inc(io_vec, 16)
            sync.dma_start(winv_sb[:], winv_d[:]).then_inc(io_w, 16)
            sync.dma_start(idx_sb[0:16, c0:cA], idx_d[:, c0:cA]).then_inc(
                r1, 16)
            # replicate 16 -> 128 partitions by doubling (ucode reads the
            # wrapped idx replicated x8)
            for rng, sem in (((0, c0), r0), ((c0, cA), r1)):
                a, b = rng
                sync.wait_ge(sem, 16)
                sync.dma_start(idx_sb[16:32, a:b],
                               idx_sb[0:16, a:b]).then_inc(sem, 16)
                sync.wait_ge(sem, 32)
                sync.dma_start(idx_sb[32:64, a:b],
                               idx_sb[0:32, a:b]).then_inc(sem, 16)
                sync.wait_ge(sem, 48)
                sync.dma_start(idx_sb[64:128, a:b],
                               idx_sb[0:64, a:b]).then_inc(sem, 16)
            for tau in range(reps * n_tiles):
                t = tau % n_tiles
                sync.wait_ge(scd, tau + 1)
                sync.dma_start(out_d[t * P:(t + 1) * P, :],
                               OSB[tau % DEPTH][:]).then_inc(
                                   sto[tau % DEPTH], 16)

        @block.gpsimd
        def _(gpsimd: bass.BassGpSimd):
            gpsimd.load_library(mlp)
            gpsimd.wait_ge(r0, 64)
            for tau in range(reps * n_tiles):
                t = tau % n_tiles
                if tau == split:
                    gpsimd.wait_ge(r1, 64)
                if tau >= DEPTH:
                    gpsimd.wait_ge(mmd, tau - DEPTH + 1)  # G[tau%DEPTH] free
                gt = G[tau % DEPTH]
                for c in range(n_chunks):
                    src = ftab[c * chunk_rows:(c + 1) * chunk_rows, :]
                    idxs = idx_sb[:, (t * n_chunks + c) * ic:
                                  (t * n_chunks + c + 1) * ic]
                    gpsimd.dma_gather(
                        gt[:, c * bpc:(c + 1) * bpc, :], src, idxs,
                        cap, int(counts[t * n_chunks + c]), d, queue_num=c,
                    ).then_inc(gq[c][tau % DEPTH], 16)

        @block.vector
        def _(vector: bass.BassVectorEngine):
            vector.wait_ge(io_vec, 32)  # b + iota landed
            iv = iota_sb.ap()
            iota_bc = bass.AP(iv.tensor, iv.offset,
                              [iv.ap[0], [0, nblk], iv.ap[1]])
            for tau in range(reps * n_tiles):
                t = tau % n_tiles
                if tau >= DEPTH:
                    vector.wait_ge(mmd, tau - DEPTH + 1)  # sel free
                st = SEL[tau % DEPTH]
                bv = b_sb[:, t * nblk:(t + 1) * nblk]
                b_bc = bass.AP(bv.tensor, bv.offset,
                               [bv.ap[0], bv.ap[1], [0, P]])
                vector.tensor_tensor(
                    out=st[:], in0=iota_bc, in1=b_bc,
                    op=mybir.AluOpType.is_equal,
                ).then_inc(selg, 1)

        @block.scalar
        def _(scalar: bass.BassEngine):
            scalar.wait_ge(io_w, 16)
            for tau in range(reps * n_tiles):
                t = tau % n_tiles
                scalar.wait_ge(mmd, tau + 1)     # psum[tau%DEPTH] ready
                if tau >= DEPTH:
                    scalar.wait_ge(sto[tau % DEPTH],
                                   16 * (tau // DEPTH))  # OSB free
                scalar.mul(OSB[tau % DEPTH][:], PS[tau % DEPTH][:],
                           winv_sb[:, t:t + 1]).then_inc(scd, 1)

        @block.tensor
        def _(tensor: bass.BassEngine):
            for tau in range(reps * n_tiles):
                for c in range(n_chunks):
                    tensor.wait_ge(gq[c][tau % DEPTH],
                                   16 * (tau // DEPTH + 1))
                tensor.wait_ge(selg, tau + 1)
                if tau >= DEPTH:
                    tensor.wait_ge(scd, tau - DEPTH + 1)  # psum drained
                gt, st, pst = (G[tau % DEPTH], SEL[tau % DEPTH],
                               PS[tau % DEPTH])
                for blk in range(nblk):
                    inst = nc.tensor.matmul(
                        pst[:], st[:, blk, :], gt[:, blk, :],
                        start=(blk == 0), stop=(blk == nblk - 1),
                    )
                inst.then_inc(mmd, 1)

    nc.compile()
    return nc


def get_program(counts, reps=1):
    key = (N_NODES, CHUNK_ROWS, N_CHUNKS, D_FEAT, TILES_PER_CORE, CAP,
           counts.tobytes(), reps)
    if key not in _prog_cache:
        _prog_cache[key] = _build_program(
            N_NODES, CHUNK_ROWS, N_CHUNKS, D_FEAT, TILES_PER_CORE, CAP,
            counts, reps=reps)
    return _prog_cache[key]


def _group_slots(midx, mask, n_tiles, chunk_rows, n_chunks):
    """Sort one core's unmasked slots by (tile, chunk); return flat arrays
    plus group start/end offsets."""
    bb, kk = np.nonzero(mask)
    gidx = midx[bb, kk]
    tile = bb // P
    lane = bb % P
    chunk = gidx // chunk_rows
    local = (gidx % chunk_rows).astype(np.int16)
    order = np.lexsort((chunk, tile))
    lane, chunk, local = lane[order], chunk[order], local[order]
    gkey = tile[order] * n_chunks + chunk
    starts = np.searchsorted(gkey, np.arange(n_tiles * n_chunks))
    ends = np.searchsorted(gkey, np.arange(n_tiles * n_chunks) + 1)
    return local, lane, starts, ends


def pack_core(local, lane, starts, ends, counts, n_tiles, cap, n_chunks):
    """Build one core's gather-index / selection-lane arrays with the
    cross-core-uniform per-group counts."""
    import ml_dtypes
    bpc = cap // P
    nblk = n_chunks * bpc
    ic = cap // 16

    idx_arr = np.zeros((16, n_tiles * n_chunks * ic), np.int16)
    b_arr = np.full((P, n_tiles * nblk), P, np.uint8)

    for g in range(n_tiles * n_chunks):
        s, e = starts[g], ends[g]
        n = e - s
        # The SWDGE ring accounting derived from num_idxs_reg (= counts[g],
        # the max across cores) only depends on ceil(count/128), so each
        # core may trim to its own exact count as long as it stays in the
        # same 128-bucket: pad up to the bucket floor with dummy valid
        # slots (row 0, dead lane), then -1 (skipped by the DMA ucode).
        # The first DEPTH tiles gather the full uniform count so the G
        # ring buffers are completely initialized.
        bucket_floor = ((int(counts[g]) + 127) // 128 - 1) * 128 + 1
        if g < DEPTH * n_chunks:
            cnt = int(counts[g])
        else:
            cnt = max(n, bucket_floor)
        assert n <= cnt <= int(counts[g]) <= cap
        flat_idx = np.full(cap, -1, np.int16)
        flat_idx[:cnt] = 0
        flat_idx[:n] = local[s:e]
        flat_b = np.full(cap, float(P), np.float32)
        flat_b[:n] = lane[s:e]
        # wrapped int16 layout: flat j -> [j%16, j//16]; the x8 partition
        # replication happens on device
        idx_arr[:, g * ic:(g + 1) * ic] = flat_idx.reshape(ic, 16).T
        # selection lane values: flat j -> block j//128, partition j%128
        t, c = g // n_chunks, g % n_chunks
        cols = flat_b.reshape(bpc, P).T          # [P, bpc]
        b_arr[:, t * nblk + c * bpc:(t * nblk + (c + 1) * bpc)] = cols
    return idx_arr, b_arr


def prep_inputs(features, neigh_idx, neigh_mask):
    import ml_dtypes
    features = np.ascontiguousarray(
        np.asarray(features, dtype=np.float32).astype(ml_dtypes.bfloat16))
    neigh_idx = np.asarray(neigh_idx).astype(np.int64)
    neigh_mask = np.asarray(neigh_mask).astype(bool)

    winv = (1.0 / np.maximum(neigh_mask.sum(-1), 1)).astype(np.float32)

    pad = B_PAD - BATCH
    midx = np.concatenate(
        [neigh_idx, np.zeros((pad, K), np.int64)], axis=0).astype(np.int32)
    mask = np.concatenate([neigh_mask, np.zeros((pad, K), bool)], axis=0)
    winv = np.concatenate([winv, np.ones(pad, np.float32)])

    iota = np.tile(np.arange(P, dtype=np.uint8), (P, 1))

    n_groups = TILES_PER_CORE * N_CHUNKS
    grouped = []
    counts = np.zeros(n_groups, np.int32)
    for c in range(N_CORES):
        sl = slice(c * B_LOC, (c + 1) * B_LOC)
        local, lane, starts, ends = _group_slots(
            midx[sl], mask[sl], TILES_PER_CORE, CHUNK_ROWS, N_CHUNKS)
        grouped.append((local, lane, starts, ends))
        counts = np.maximum(counts, (ends - starts).astype(np.int32))
    # num_idxs_reg must be >= 1 (a zero-index gather wedges the ucode), and
    # the first DEPTH tiles must gather a full cap so the G ring buffers
    # never expose uninitialized SBUF.
    counts = np.maximum(counts, 1)
    counts[:DEPTH * N_CHUNKS] = CAP
    assert counts.max() <= CAP

    in_maps = []
    for c in range(N_CORES):
        local, lane, starts, ends = grouped[c]
        idx_arr, b_arr = pack_core(local, lane, starts, ends, counts,
                                   TILES_PER_CORE, CAP, N_CHUNKS)
        sl = slice(c * B_LOC, (c + 1) * B_LOC)
        winv_arr = np.ascontiguousarray(
            winv[sl].reshape(TILES_PER_CORE, P).T.astype(np.float32))
        in_maps.append({
            "features": features,
            "idx": idx_arr,
            "bidx": b_arr,
            "iota": iota,
            "winv": winv_arr,
        })
    return in_maps, counts


def kernel(features, neigh_idx, neigh_mask):
    from concourse.bass_utils import run_bass_kernel_spmd

    in_maps, counts = prep_inputs(features, neigh_idx, neigh_mask)
    nc = get_program(counts)
    res = run_bass_kernel_spmd(nc, in_maps, list(range(N_CORES)))
    full = np.concatenate(
        [res.results[c]["out"].astype(np.float32) for c in range(N_CORES)],
        axis=0)
    return full[:BATCH]


# revision 35
# speedup vs baseline: 1.9702x; 1.8133x over previous
"""Masked mean neighbor aggregation (GNN message passing) on 8 TRN2 cores.

Strategy (per spec sharding hint): batch dim sharded across 8 cores, feature
table replicated in each core's DRAM (converted to bf16 on the host, which
halves gather traffic and runs the PE at bf16 rate).

Device algorithm, per core, per 128-row output tile:
  - The table is split into 4 row-range chunks (<=32768 rows each) so row
    indices fit dma_gather's int16 index format.  The host compacts each
    tile's unmasked (row, neighbor) slots by chunk into fixed-capacity flat
    index lists.
  - 4 dma_gather instructions (one per chunk, on 4 parallel SWDGE queues)
    pull the 256 B bf16 feature rows into an SBUF tile G.  Trailing slots
    are -1 (dma_gather skips them), so only the group's real slot count is
    transferred.  The SWDGE ring accounting is computed from num_idxs_reg,
    which must equal the exact count of non-negative indices and is baked
    into the instruction stream -- so the count of each (tile, chunk) group
    is made uniform across the 8 SPMD cores by padding up to the max with
    dummy valid slots (row 0, dead selection lane).
  - The slots land in a data-dependent order, so the host also ships a tiny
    per-slot target-lane array; the vector engine expands it on device into
    one-hot bf16 selection matrices (pad slots get lane 128 -> all-zero
    column).
  - PE computes psum[b, d] = sum_slots sel[slot, b] * G[slot, d] with PSUM
    accumulation over the 16 slot-blocks: simultaneously the masked sum and
    the reordering.
  - The scalar engine scales by 1/max(count,1) (host-precomputed) and the
    f32 result is stored.

The first DEPTH tiles gather a full cap so the G ring buffers never expose
uninitialized SBUF (stale NaN * 0 = NaN in PSUM).  Everything is raw bacc
with manual semaphores (the Tile layer does not know dma_gather's DMA
semantics).
"""

from contextlib import ExitStack

import numpy as np

N_NODES = 100000
D_FEAT = 128
BATCH = 50000
K = 25
N_CORES = 8
P = 128

N_CHUNKS = 4
CHUNK_ROWS = 25000           # N_NODES / N_CHUNKS, < 32768 so int16-safe
CAP = 512                    # per (tile, chunk) gather capacity, mult of 128
TILES_PER_CORE = 49          # ceil(50000 / 8 / 128)
B_LOC = TILES_PER_CORE * P   # 6272
B_PAD = B_LOC * N_CORES      # 50176
DEPTH = 8                    # ring depth; first DEPTH tiles gather full cap

_prog_cache = {}


def _build_program(n_rows, chunk_rows, n_chunks, d, n_tiles, cap, counts,
                   reps=1):
    import concourse.bass as bass
    import concourse.bacc as bacc
    import concourse.mybir as mybir
    from concourse.library_config import mlp

    bpc = cap // P               # G column-blocks per chunk
    nblk = n_chunks * bpc        # selection blocks per tile
    ic = cap // 16               # idx columns per gather (wrapped int16)

    nc = bacc.Bacc("TRN2", target_bir_lowering=False, debug=False,
                   num_devices=N_CORES, num_swdge_queues=n_chunks)

    ftab = nc.dram_tensor("features", [n_rows, d], mybir.dt.bfloat16,
                          kind="ExternalInput")
    # idx ships unreplicated (16 partitions); the ucode wants it replicated
    # x8 across 128 partitions, which is done on device with 3 doubling
    # SBUF->SBUF copies (saves 1.4 MB of HBM reads per pass).
    idx_d = nc.dram_tensor("idx", [16, n_tiles * n_chunks * ic],
                           mybir.dt.int16, kind="ExternalInput")
    b_d = nc.dram_tensor("bidx", [P, n_tiles * nblk], mybir.dt.uint8,
                         kind="ExternalInput")
    iota_d = nc.dram_tensor("iota", [P, P], mybir.dt.uint8,
                            kind="ExternalInput")
    winv_d = nc.dram_tensor("winv", [P, n_tiles], mybir.dt.float32,
                            kind="ExternalInput")
    out_d = nc.dram_tensor("out", [n_tiles * P, d], mybir.dt.bfloat16,
                           kind="ExternalOutput")

    with ExitStack() as stack:
        block = stack.enter_context(nc.Block())
        ec = stack.enter_context
        idx_sb = ec(nc.sbuf_tensor("idx_sb", [P, n_tiles * n_chunks * ic],
                                   mybir.dt.int16))
        b_sb = ec(nc.sbuf_tensor("b_sb", [P, n_tiles * nblk],
                                 mybir.dt.uint8))
        iota_sb = ec(nc.sbuf_tensor("iota_sb", [P, P], mybir.dt.uint8))
        winv_sb = ec(nc.sbuf_tensor("winv_sb", [P, n_tiles],
                                    mybir.dt.float32))
        G = [ec(nc.sbuf_tensor(f"g{r}", [P, nblk, d], mybir.dt.bfloat16))
             for r in range(DEPTH)]
        SEL = [ec(nc.sbuf_tensor(f"sel{r}", [P, nblk, P], mybir.dt.bfloat16))
               for r in range(DEPTH)]
        OSB = [ec(nc.sbuf_tensor(f"osb{r}", [P, d], mybir.dt.bfloat16))
               for r in range(DEPTH)]
        PS = [ec(nc.psum_tensor(f"ps{r}", [P, d], mybir.dt.float32))
              for r in range(DEPTH)]
        r0 = ec(nc.semaphore("r0"))
        r1 = ec(nc.semaphore("r1"))
        io_vec = ec(nc.semaphore("io_vec"))
        io_w = ec(nc.semaphore("io_w"))
        gq = [[ec(nc.semaphore(f"gq{c}_{r}")) for r in range(DEPTH)]
              for c in range(n_chunks)]
        selg = ec(nc.semaphore("selg"))
        mmd = ec(nc.semaphore("mmd"))
        scd = ec(nc.semaphore("scd"))
        sto = [ec(nc.semaphore(f"sto{r}")) for r in range(DEPTH)]

        split = min(DEPTH, n_tiles)
        c0 = split * n_chunks * ic          # idx cols for the first tiles
        cA = n_tiles * n_chunks * ic

        @block.sync
        def _(sync: bass.BassEngine):
            # idx piece for the first `split` tiles: 8 independent direct
            # HBM reads into the 8 partition groups -- latency-parallel, so
            # gathers start ~1 us into the pass (the piece is only 33 KB,
            # the x8 re-read is cheap here)
            for r in range(8):
                sync.dma_start(idx_sb[16 * r:16 * (r + 1), 0:c0],
                               idx_d[:, 0:c0]).then_inc(r0, 16)
            sync.dma_start(b_sb[:], b_d[:]).then_inc(io_vec, 16)
            sync.dma_start(iota_sb[:], iota_d[:]).then_inc(io_vec, 16)
            sync.dma_start(winv_sb[:], winv_d[:]).then_inc(io_w, 16)
            # bulk idx piece: one HBM read + 3 doubling SBUF->SBUF copies
            # (saves 7/8 of the HBM bytes; the serial chain's latency hides
            # behind the first `split` tiles' execution)
            sync.dma_start(idx_sb[0:16, c0:cA], idx_d[:, c0:cA]).then_inc(
                r1, 16)
            sync.wait_ge(r1, 16)
            sync.dma_start(idx_sb[16:32, c0:cA],
                           idx_sb[0:16, c0:cA]).then_inc(r1, 16)
            sync.wait_ge(r1, 32)
            sync.dma_start(idx_sb[32:64, c0:cA],
                           idx_sb[0:32, c0:cA]).then_inc(r1, 16)
            sync.wait_ge(r1, 48)
            sync.dma_start(idx_sb[64:128, c0:cA],
                           idx_sb[0:64, c0:cA]).then_inc(r1, 16)
            for tau in range(reps * n_tiles):
                t = tau % n_tiles
                sync.wait_ge(scd, tau + 1)
                sync.dma_start(out_d[t * P:(t + 1) * P, :],
                               OSB[tau % DEPTH][:]).then_inc(
                                   sto[tau % DEPTH], 16)

        @block.gpsimd
        def _(gpsimd: bass.BassGpSimd):
            gpsimd.load_library(mlp)
            gpsimd.wait_ge(r0, 128)     # 8 direct-replicated piece-0 DMAs
            for tau in range(reps * n_tiles):
                t = tau % n_tiles
                if tau == split:
                    gpsimd.wait_ge(r1, 64)
                if tau >= DEPTH:
                    gpsimd.wait_ge(mmd, tau - DEPTH + 1)  # G[tau%DEPTH] free
                gt = G[tau % DEPTH]
                for c in range(n_chunks):
                    src = ftab[c * chunk_rows:(c + 1) * chunk_rows, :]
                    idxs = idx_sb[:, (t * n_chunks + c) * ic:
                                  (t * n_chunks + c + 1) * ic]
                    gpsimd.dma_gather(
                        gt[:, c * bpc:(c + 1) * bpc, :], src, idxs,
                        cap, int(counts[t * n_chunks + c]), d, queue_num=c,
                    ).then_inc(gq[c][tau % DEPTH], 16)

        @block.vector
        def _(vector: bass.BassVectorEngine):
            vector.wait_ge(io_vec, 32)  # b + iota landed
            iv = iota_sb.ap()
            iota_bc = bass.AP(iv.tensor, iv.offset,
                              [iv.ap[0], [0, nblk], iv.ap[1]])
            for tau in range(reps * n_tiles):
                t = tau % n_tiles
                if tau >= DEPTH:
                    vector.wait_ge(mmd, tau - DEPTH + 1)  # sel free
                st = SEL[tau % DEPTH]
                bv = b_sb[:, t * nblk:(t + 1) * nblk]
                b_bc = bass.AP(bv.tensor, bv.offset,
                               [bv.ap[0], bv.ap[1], [0, P]])
                vector.tensor_tensor(
                    out=st[:], in0=iota_bc, in1=b_bc,
                    op=mybir.AluOpType.is_equal,
                ).then_inc(selg, 1)

        @block.scalar
        def _(scalar: bass.BassEngine):
            scalar.wait_ge(io_w, 16)
            for tau in range(reps * n_tiles):
                t = tau % n_tiles
                scalar.wait_ge(mmd, tau + 1)     # psum[tau%DEPTH] ready
                if tau >= DEPTH:
                    scalar.wait_ge(sto[tau % DEPTH],
                                   16 * (tau // DEPTH))  # OSB free
                scalar.mul(OSB[tau % DEPTH][:], PS[tau % DEPTH][:],
                           winv_sb[:, t:t + 1]).then_inc(scd, 1)

        @block.tensor
        def _(tensor: bass.BassEngine):
            for tau in range(reps * n_tiles):
                for c in range(n_chunks):
                    tensor.wait_ge(gq[c][tau % DEPTH],
                                   16 * (tau // DEPTH + 1))
                tensor.wait_ge(selg, tau + 1)
                if tau >= DEPTH:
                    tensor.wait_ge(scd, tau - DEPTH + 1)  # psum drained
                gt, st, pst = (G[tau % DEPTH], SEL[tau % DEPTH],
                               PS[tau % DEPTH])
                for blk in range(nblk):
                    inst = nc.tensor.matmul(
                        pst[:], st[:, blk, :], gt[:, blk, :],
                        start=(blk == 0), stop=(blk == nblk - 1),
                    )
                inst.then_inc(mmd, 1)

    nc.compile()
    return nc


def get_program(counts, reps=1):
    key = (N_NODES, CHUNK_ROWS, N_CHUNKS, D_FEAT, TILES_PER_CORE, CAP,
           counts.tobytes(), reps)
    if key not in _prog_cache:
        _prog_cache[key] = _build_program(
            N_NODES, CHUNK_ROWS, N_CHUNKS, D_FEAT, TILES_PER_CORE, CAP,
            counts, reps=reps)
    return _prog_cache[key]


def _group_slots(midx, mask, n_tiles, chunk_rows, n_chunks):
    """Sort one core's unmasked slots by (tile, chunk); return flat arrays
    plus group start/end offsets."""
    bb, kk = np.nonzero(mask)
    gidx = midx[bb, kk]
    tile = bb // P
    lane = bb % P
    chunk = gidx // chunk_rows
    local = (gidx % chunk_rows).astype(np.int16)
    order = np.lexsort((chunk, tile))
    lane, chunk, local = lane[order], chunk[order], local[order]
    gkey = tile[order] * n_chunks + chunk
    starts = np.searchsorted(gkey, np.arange(n_tiles * n_chunks))
    ends = np.searchsorted(gkey, np.arange(n_tiles * n_chunks) + 1)
    return local, lane, starts, ends


def pack_core(local, lane, starts, ends, counts, n_tiles, cap, n_chunks):
    """Build one core's gather-index / selection-lane arrays with the
    cross-core-uniform per-group counts."""
    import ml_dtypes
    bpc = cap // P
    nblk = n_chunks * bpc
    ic = cap // 16

    idx_arr = np.zeros((16, n_tiles * n_chunks * ic), np.int16)
    b_arr = np.full((P, n_tiles * nblk), P, np.uint8)

    for g in range(n_tiles * n_chunks):
        s, e = starts[g], ends[g]
        n = e - s
        # The SWDGE ring accounting derived from num_idxs_reg (= counts[g],
        # the max across cores) only depends on ceil(count/128), so each
        # core may trim to its own exact count as long as it stays in the
        # same 128-bucket: pad up to the bucket floor with dummy valid
        # slots (row 0, dead lane), then -1 (skipped by the DMA ucode).
        # The first DEPTH tiles gather the full uniform count so the G
        # ring buffers are completely initialized.
        bucket_floor = ((int(counts[g]) + 127) // 128 - 1) * 128 + 1
        if g < DEPTH * n_chunks:
            cnt = int(counts[g])
        else:
            cnt = max(n, bucket_floor)
        assert n <= cnt <= int(counts[g]) <= cap
        flat_idx = np.full(cap, -1, np.int16)
        flat_idx[:cnt] = 0
        flat_idx[:n] = local[s:e]
        flat_b = np.full(cap, float(P), np.float32)
        flat_b[:n] = lane[s:e]
        # wrapped int16 layout: flat j -> [j%16, j//16]; the x8 partition
        # replication happens on device
        idx_arr[:, g * ic:(g + 1) * ic] = flat_idx.reshape(ic, 16).T
        # selection lane values: flat j -> block j//128, partition j%128
        t, c = g // n_chunks, g % n_chunks
        cols = flat_b.reshape(bpc, P).T          # [P, bpc]
        b_arr[:, t * nblk + c * bpc:(t * nblk + (c + 1) * bpc)] = cols
    return idx_arr, b_arr


def prep_inputs(features, neigh_idx, neigh_mask):
    import ml_dtypes
    features = np.ascontiguousarray(
        np.asarray(features, dtype=np.float32).astype(ml_dtypes.bfloat16))
    neigh_idx = np.asarray(neigh_idx).astype(np.int64)
    neigh_mask = np.asarray(neigh_mask).astype(bool)

    winv = (1.0 / np.maximum(neigh_mask.sum(-1), 1)).astype(np.float32)

    pad = B_PAD - BATCH
    midx = np.concatenate(
        [neigh_idx, np.zeros((pad, K), np.int64)], axis=0).astype(np.int32)
    mask = np.concatenate([neigh_mask, np.zeros((pad, K), bool)], axis=0)
    winv = np.concatenate([winv, np.ones(pad, np.float32)])

    iota = np.tile(np.arange(P, dtype=np.uint8), (P, 1))

    n_groups = TILES_PER_CORE * N_CHUNKS
    grouped = []
    counts = np.zeros(n_groups, np.int32)
    for c in range(N_CORES):
        sl = slice(c * B_LOC, (c + 1) * B_LOC)
        local, lane, starts, ends = _group_slots(
            midx[sl], mask[sl], TILES_PER_CORE, CHUNK_ROWS, N_CHUNKS)
        grouped.append((local, lane, starts, ends))
        counts = np.maximum(counts, (ends - starts).astype(np.int32))
    # num_idxs_reg must be >= 1 (a zero-index gather wedges the ucode), and
    # the first DEPTH tiles must gather a full cap so the G ring buffers
    # never expose uninitialized SBUF.
    counts = np.maximum(counts, 1)
    counts[:DEPTH * N_CHUNKS] = CAP
    assert counts.max() <= CAP

    in_maps = []
    for c in range(N_CORES):
        local, lane, starts, ends = grouped[c]
        idx_arr, b_arr = pack_core(local, lane, starts, ends, counts,
                                   TILES_PER_CORE, CAP, N_CHUNKS)
        sl = slice(c * B_LOC, (c + 1) * B_LOC)
        winv_arr = np.ascontiguousarray(
            winv[sl].reshape(TILES_PER_CORE, P).T.astype(np.float32))
        in_maps.append({
            "features": features,
            "idx": idx_arr,
            "bidx": b_arr,
            "iota": iota,
            "winv": winv_arr,
        })
    return in_maps, counts


def kernel(features, neigh_idx, neigh_mask):
    from concourse.bass_utils import run_bass_kernel_spmd

    in_maps, counts = prep_inputs(features, neigh_idx, neigh_mask)
    nc = get_program(counts)
    res = run_bass_kernel_spmd(nc, in_maps, list(range(N_CORES)))
    full = np.concatenate(
        [res.results[c]["out"].astype(np.float32) for c in range(N_CORES)],
        axis=0)
    return full[:BATCH]
